# revision 7
# baseline (speedup 1.0000x reference)
"""Trainium2 Bass kernel for nn_DenoisingConditionDecoder.

Per-core computation (data-parallel over batch, 1 batch element per core):
  gate  = sigmoid([nx, cond] @ W_gate + b_gate)
  fused = gate*nx + (1-gate)*cond
  attn  = softmax(fused @ X^T / sqrt(D)) @ X
  q     = LN(fused + attn) * g1 + be1
  ff    = gelu(q @ W1 + b1) @ W2 + b2
  out   = LN(q + ff) * g2 + be2

Layout strategy: activations feeding matmul contractions live transposed
([feat, seq], "T" layout) so the contraction dim sits on partitions;
softmax-normalize / layernorm / residuals run in natural [seq, feat] layout.
PE-transposes bridge the two. Matmul operands are bf16 (fp32 PSUM accum).

Attention uses a scores-transposed formulation: scoresT[k, q] tiles come out
of the PE with k on partitions, exp runs on ACT into bf16 tiles which are
directly the lhsT (stationary) operand of attn_out = attn @ X — no [S,S]
transpose needed.  Row sums for softmax ride as an extra N=1 matmul against
a ones vector into a [q, 1] PSUM, giving per-partition reciprocals.

SBUF is statically allocated per (pool, tag); later-phase tensors reuse
earlier tags of identical geometry (qT<-condT slots, exp<-nxmcT slots,
w2<-wg slots, ff1<-XN slots, out<-stg slots).
"""

import math
import numpy as np

import concourse.bass as bass
import concourse.tile as tile
from concourse import bacc, mybir
from concourse.bass_utils import run_bass_kernel_spmd
from concourse.masks import make_identity

B, S, D = 8, 2048, 512
H = 2 * D
P = 128
NT = S // P   # 16 seq tiles
DT = D // P   # 4 feature tiles
HT = H // P   # 8 hidden tiles
QB = 512      # q-block (moving-dim chunk)
NQB = S // QB # 4
JB = QB // P  # 4 q-subtiles per block
LN_EPS = 1e-5
SCALE = 1.0 / math.sqrt(D)

F32 = mybir.dt.float32
BF16 = mybir.dt.bfloat16
AF = mybir.ActivationFunctionType
ALU = mybir.AluOpType

N_CORES = 8

_cache = {}


def _build(gelu_func=None):
    gelu_func = AF.Gelu if gelu_func is None else gelu_func
    nc = bacc.Bacc("TRN2", target_bir_lowering=False, debug=False,
                   num_devices=N_CORES)

    nx_d = nc.dram_tensor("nx", [S, D], F32, kind="ExternalInput")
    x_d = nc.dram_tensor("x", [S, D], F32, kind="ExternalInput")
    cond_d = nc.dram_tensor("cond", [S, D], F32, kind="ExternalInput")
    wg_d = nc.dram_tensor("wg", [H, D], F32, kind="ExternalInput")
    bg_d = nc.dram_tensor("bg", [D], F32, kind="ExternalInput")
    w1_d = nc.dram_tensor("w1", [D, H], F32, kind="ExternalInput")
    b1_d = nc.dram_tensor("b1", [H], F32, kind="ExternalInput")
    w2_d = nc.dram_tensor("w2", [H, D], F32, kind="ExternalInput")
    b2_d = nc.dram_tensor("b2", [D], F32, kind="ExternalInput")
    g1_d = nc.dram_tensor("g1", [D], F32, kind="ExternalInput")
    be1_d = nc.dram_tensor("be1", [D], F32, kind="ExternalInput")
    g2_d = nc.dram_tensor("g2", [D], F32, kind="ExternalInput")
    be2_d = nc.dram_tensor("be2", [D], F32, kind="ExternalInput")
    out_d = nc.dram_tensor("out", [S, D], F32, kind="ExternalOutput")

    with tile.TileContext(nc) as tc:
        _body(nc, tc, nx_d, x_d, cond_d, wg_d, bg_d, w1_d, b1_d, w2_d, b2_d,
              g1_d, be1_d, g2_d, be2_d, out_d, gelu_func)
    nc.compile()
    return nc


def _body(nc, tc, nx_d, x_d, cond_d, wg_d, bg_d, w1_d, b1_d, w2_d, b2_d,
          g1_d, be1_d, g2_d, be2_d, out_d, gelu_func):
    from contextlib import ExitStack

    ctx = ExitStack()
    with ctx:
        # ---------------- pools ----------------
        const = ctx.enter_context(tc.tile_pool(name="const", bufs=1))
        p_stg = ctx.enter_context(tc.tile_pool(name="stg", bufs=6))
        p_w = ctx.enter_context(tc.tile_pool(name="w", bufs=1))
        p_big = ctx.enter_context(tc.tile_pool(name="big", bufs=4))   # [P,S]
        p_seq = ctx.enter_context(tc.tile_pool(name="seq", bufs=16))  # [P,D]
        p_gate = ctx.enter_context(tc.tile_pool(name="gatep", bufs=3))
        p_ff2 = ctx.enter_context(tc.tile_pool(name="ff2p", bufs=3))
        p_r2 = ctx.enter_context(tc.tile_pool(name="r2p", bufs=4))
        p_sm = ctx.enter_context(tc.tile_pool(name="sm", bufs=8))
        p_xh = ctx.enter_context(tc.tile_pool(name="xh", bufs=3))

        ps_mm = ctx.enter_context(tc.tile_pool(name="psmm", bufs=3,
                                               space="PSUM"))
        ps_tr = ctx.enter_context(tc.tile_pool(name="pstr", bufs=3,
                                               space="PSUM"))
        ps_n = ctx.enter_context(tc.tile_pool(name="psn", bufs=2,
                                              space="PSUM"))

        # ---------------- constants ----------------
        ident_f = const.tile([P, P], F32, tag="idf")
        make_identity(nc, ident_f)
        ident_b = const.tile([P, P], BF16, tag="idb")
        make_identity(nc, ident_b)
        ones_b = const.tile([P, 1], BF16, tag="ones")
        nc.vector.memset(ones_b, 1.0)
        eps_t = const.tile([P, 1], F32, tag="eps")
        nc.vector.memset(eps_t, LN_EPS)

        def bcast_vec(dram, tag):
            t = const.tile([P, D], F32, tag=tag)
            a = dram.ap()
            src = bass.AP(tensor=a.tensor, offset=a.offset,
                          ap=[[0, P]] + list(a.ap))
            nc.sync.dma_start(out=t, in_=src)
            return t

        g1b = bcast_vec(g1_d, "g1")
        be1b = bcast_vec(be1_d, "be1")
        g2b = bcast_vec(g2_d, "g2")
        be2b = bcast_vec(be2_d, "be2")

        def part_vec(dram, n, tag):
            ts = []
            for m in range(n):
                t = p_sm.tile([P, 1], F32, tag=tag, bufs=n, name=f"{tag}{m}")
                nc.sync.dma_start(
                    out=t, in_=dram.ap()[m * P:(m + 1) * P].unsqueeze(1))
                ts.append(t)
            return ts

        bg_sb = part_vec(bg_d, DT, "bg")
        b1_sb = part_vec(b1_d, HT, "b1")
        b2_sb = part_vec(b2_d, DT, "b2")

        # ---------------- weights (cast f32 -> bf16 during DMA) ----------
        def load_w(dram, n, cols, tag, pref):
            ts = []
            for k in range(n):
                t = p_w.tile([P, cols], BF16, tag=tag, bufs=n,
                             name=f"{pref}{k}")
                nc.gpsimd.dma_start(out=t, in_=dram.ap()[k * P:(k + 1) * P, :])
                ts.append(t)
            return ts

        wg_b = load_w(wg_d, HT, D, "wg", "wg")
        w1_b = load_w(w1_d, DT, H, "w1", "w1")
        # gate uses combined=[nx,cond]; rewrite with nxmc=(nx-cond):
        #   logits = nxmc @ Wg_top + cond @ (Wg_top + Wg_bot)
        for k in range(DT):
            nc.vector.tensor_add(wg_b[k + DT], wg_b[k + DT], wg_b[k])

        # ---------------- stage 1: load + build T-layout inputs ----------
        condT = [p_big.tile([P, S], BF16, tag="condT", bufs=DT,
                            name=f"condT{j}") for j in range(DT)]
        nxmcT = [p_big.tile([P, S], BF16, tag="nxmcT", bufs=DT,
                            name=f"nxmcT{j}") for j in range(DT)]
        XT = [p_big.tile([P, S], BF16, tag="XT", bufs=DT,
                         name=f"XT{j}") for j in range(DT)]
        XN = [p_seq.tile([P, D], BF16, tag="XN", bufs=NT,
                         name=f"XN{j}") for j in range(NT)]

        for i in range(NT):
            row = slice(i * P, (i + 1) * P)
            nx_s = p_stg.tile([P, D], F32, tag="stg", name=f"nxs{i}")
            nc.sync.dma_start(out=nx_s, in_=nx_d.ap()[row, :])
            cond_s = p_stg.tile([P, D], F32, tag="stg", name=f"cds{i}")
            nc.sync.dma_start(out=cond_s, in_=cond_d.ap()[row, :])
            x_s = p_stg.tile([P, D], F32, tag="stg", name=f"xs{i}")
            nc.sync.dma_start(out=x_s, in_=x_d.ap()[row, :])
            nc.any.tensor_copy(out=XN[i], in_=x_s)
            sub_s = p_stg.tile([P, D], F32, tag="stg", name=f"sbs{i}")
            nc.vector.tensor_sub(sub_s, nx_s, cond_s)
            for j in range(DT):
                col = slice(j * P, (j + 1) * P)
                pt = ps_tr.tile([P, P], F32, tag="tr", name=f"trc{i}_{j}")
                nc.tensor.transpose(pt, cond_s[:, col], ident_f)
                nc.any.tensor_copy(out=condT[j][:, row], in_=pt)
                pt2 = ps_tr.tile([P, P], F32, tag="tr", name=f"trn{i}_{j}")
                nc.tensor.transpose(pt2, sub_s[:, col], ident_f)
                nc.any.tensor_copy(out=nxmcT[j][:, row], in_=pt2)
                pt3 = ps_tr.tile([P, P], BF16, tag="tr", name=f"trx{i}_{j}")
                nc.tensor.transpose(pt3, XN[i][:, col], ident_b)
                nc.any.tensor_copy(out=XT[j][:, row], in_=pt3)

        # ---------------- stage 2: gate matmul + fusion -------------------
        fusedT = [p_big.tile([P, S], BF16, tag="fT", bufs=DT,
                             name=f"fusedT{j}") for j in range(DT)]
        fusedN = [p_seq.tile([P, D], BF16, tag="fN", bufs=NT,
                             name=f"fusedN{j}") for j in range(NT)]
        qN = [p_seq.tile([P, D], F32, tag="qN", bufs=NT,
                         name=f"qN{j}") for j in range(NT)]

        for m in range(DT):
            mcol = slice(m * P, (m + 1) * P)
            for qc in range(NQB):
                qs = slice(qc * QB, (qc + 1) * QB)
                ps = ps_mm.tile([P, QB], F32, tag="mm", name=f"psg{m}_{qc}")
                for k in range(HT):
                    src = nxmcT[k] if k < DT else condT[k - DT]
                    nc.tensor.matmul(ps, wg_b[k][:, mcol], src[:, qs],
                                     start=(k == 0), stop=(k == HT - 1))
                gt = p_gate.tile([P, QB], BF16, tag="gate", name=f"gt{m}_{qc}")
                nc.scalar.activation(gt, ps, AF.Sigmoid, bias=bg_sb[m])
                # fusedT = cond + gate * (nx - cond)
                nc.vector.tensor_mul(gt, gt, nxmcT[m][:, qs])
                nc.vector.tensor_add(fusedT[m][:, qs], gt, condT[m][:, qs])
                # natural-layout fused for the residual stream
                for j in range(JB):
                    qi = qc * JB + j
                    bcol = slice(qc * QB + j * P, qc * QB + (j + 1) * P)
                    pt = ps_tr.tile([P, P], BF16, tag="tr",
                                    name=f"trf{m}_{qi}")
                    nc.tensor.transpose(pt, fusedT[m][:, bcol], ident_b)
                    nc.any.tensor_copy(out=fusedN[qi][:, mcol], in_=pt)

        # ---------------- stage 3: attention ------------------------------
        for qb in range(NQB):
            qs = slice(qb * QB, (qb + 1) * QB)
            # exp(scoresT) blocks: 4 tiles [P, S]; block b holds k-tiles
            # 4b..4b+3 in its 4 QB-wide column groups (reuses nxmcT slots)
            eblk = [p_big.tile([P, S], BF16, tag="nxmcT", bufs=DT,
                               name=f"eblk{qb}_{b}") for b in range(DT)]
            for kt in range(NT):
                ps = ps_mm.tile([P, QB], F32, tag="mm", name=f"pss{qb}_{kt}")
                for dj in range(DT):
                    nc.tensor.matmul(ps, XT[dj][:, kt * P:(kt + 1) * P],
                                     fusedT[dj][:, qs],
                                     start=(dj == 0), stop=(dj == DT - 1))
                ecol = slice((kt % DT) * QB, (kt % DT + 1) * QB)
                nc.scalar.activation(eblk[kt // DT][:, ecol], ps, AF.Exp,
                                     scale=SCALE)
            for j in range(JB):
                qi = qb * JB + j
                pa = ps_mm.tile([P, D], F32, tag="mm", name=f"psa{qi}")
                pn = ps_n.tile([P, 1], F32, tag="n", name=f"psnn{qi}")
                for kt in range(NT):
                    lhs = eblk[kt // DT][:, (kt % DT) * QB + j * P:
                                         (kt % DT) * QB + (j + 1) * P]
                    nc.tensor.matmul(pa, lhs, XN[kt],
                                     start=(kt == 0), stop=(kt == NT - 1))
                    nc.tensor.matmul(pn, lhs, ones_b,
                                     start=(kt == 0), stop=(kt == NT - 1))
                rec = p_sm.tile([P, 1], F32, tag="rec", name=f"rec{qi}")
                nc.vector.reciprocal(rec, pn)
                # r1 = attn_out/rowsum + fused   (into qN, LN1 in place)
                nc.vector.scalar_tensor_tensor(
                    qN[qi], pa, rec, fusedN[qi], ALU.mult, ALU.add)
                _layernorm(nc, p_sm, p_xh, qN[qi], g1b, be1b, qN[qi], eps_t)

        # ---------------- stage 4: q^T (reuses condT slots) ---------------
        qT = [p_big.tile([P, S], BF16, tag="condT", bufs=DT,
                         name=f"qT{j}") for j in range(DT)]
        for i in range(NT):
            row = slice(i * P, (i + 1) * P)
            qb_t = p_gate.tile([P, D], BF16, tag="gate", name=f"qc{i}")
            nc.any.tensor_copy(out=qb_t, in_=qN[i])
            for j in range(DT):
                col = slice(j * P, (j + 1) * P)
                pt = ps_tr.tile([P, P], BF16, tag="tr", name=f"trq{i}_{j}")
                nc.tensor.transpose(pt, qb_t[:, col], ident_b)
                nc.any.tensor_copy(out=qT[j][:, row], in_=pt)

        # ---------------- stage 5: FFN + LN2 + store ----------------------
        w2_b = load_w(w2_d, HT, D, "wg", "w2")  # reuses wg slots
        for qb in range(NQB):
            qs = slice(qb * QB, (qb + 1) * QB)
            ff1 = []
            for m in range(HT):
                mcol = slice(m * P, (m + 1) * P)
                ps = ps_mm.tile([P, QB], F32, tag="mm", name=f"psf{qb}_{m}")
                for k in range(DT):
                    nc.tensor.matmul(ps, w1_b[k][:, mcol], qT[k][:, qs],
                                     start=(k == 0), stop=(k == DT - 1))
                ft = p_seq.tile([P, QB], BF16, tag="XN", bufs=NT,
                                name=f"ff1_{qb}_{m}")
                nc.scalar.activation(ft, ps, gelu_func, bias=b1_sb[m])
                ff1.append(ft)
            r2 = [p_r2.tile([P, D], F32, tag="r2", name=f"r2_{qb}_{j2}")
                  for j2 in range(JB)]
            for m in range(DT):
                mcol = slice(m * P, (m + 1) * P)
                ps = ps_mm.tile([P, QB], F32, tag="mm", name=f"pso{qb}_{m}")
                for k in range(HT):
                    nc.tensor.matmul(ps, w2_b[k][:, mcol], ff1[k],
                                     start=(k == 0), stop=(k == HT - 1))
                f2 = p_ff2.tile([P, QB], BF16, tag="ff2", name=f"f2_{qb}_{m}")
                nc.scalar.activation(f2, ps, AF.Identity, bias=b2_sb[m])
                for j in range(JB):
                    qi = qb * JB + j
                    pt = ps_tr.tile([P, P], BF16, tag="tr",
                                    name=f"tro{qb}_{m}_{j}")
                    nc.tensor.transpose(pt, f2[:, j * P:(j + 1) * P], ident_b)
                    nc.vector.tensor_add(r2[j][:, mcol], pt, qN[qi][:, mcol])
            for j in range(JB):
                qi = qb * JB + j
                ot = p_stg.tile([P, D], F32, tag="stg", name=f"ot{qi}")
                _layernorm(nc, p_sm, p_xh, r2[j], g2b, be2b, ot, eps_t)
                nc.sync.dma_start(out=out_d.ap()[qi * P:(qi + 1) * P, :],
                                  in_=ot)


def _layernorm(nc, p_sm, p_xh, x, gb, bb, out, eps_t):
    """out = (x - mean)/sqrt(var+eps) * gb + bb   (row-wise over free dim)"""
    st = p_sm.tile([P, nc.vector.BN_STATS_DIM], F32, tag="bnst")
    nc.vector.bn_stats(st, x)
    mv = p_sm.tile([P, nc.vector.BN_AGGR_DIM], F32, tag="bnmv")
    nc.vector.bn_aggr(mv, st)
    sd = p_sm.tile([P, 1], F32, tag="sd")
    nc.scalar.activation(sd, mv[:, 1:2], AF.Sqrt, bias=eps_t)
    rstd = p_sm.tile([P, 1], F32, tag="rstd")
    nc.vector.reciprocal(rstd, sd)
    nmr = p_sm.tile([P, 1], F32, tag="nmr")
    nc.vector.scalar_tensor_tensor(nmr, mv[:, 0:1], -1.0, rstd,
                                   ALU.mult, ALU.mult)
    xh = p_xh.tile([P, D], F32, tag="xh")
    nc.scalar.activation(xh, x, AF.Identity, bias=nmr, scale=rstd)
    nc.vector.tensor_mul(xh, xh, gb)
    nc.vector.tensor_add(out, xh, bb)


_IN_MAP = {
    "Noise_x": "nx", "X": "x", "cond": "cond",
    "W_gate": "wg", "b_gate": "bg", "W1": "w1", "b1": "b1",
    "W2": "w2", "b2": "b2", "g1": "g1", "be1": "be1",
    "g2": "g2", "be2": "be2",
}


def _run(inputs, trace=False):
    if "nc" not in _cache:
        _cache["nc"] = _build()
    nc = _cache["nc"]

    in_maps = []
    for c in range(N_CORES):
        m = {}
        for src, dst in _IN_MAP.items():
            a = np.ascontiguousarray(np.asarray(inputs[src], dtype=np.float32))
            m[dst] = a[c] if a.ndim == 3 else a
        in_maps.append(m)
    res = run_bass_kernel_spmd(nc, in_maps, list(range(N_CORES)), trace=trace)
    out = np.stack([res.results[c]["out"] for c in range(N_CORES)], axis=0)
    return out, res


def kernel(**inputs) -> np.ndarray:
    out, _ = _run(inputs, trace=False)
    return out
